# revision 14
# baseline (speedup 1.0000x reference)
"""Trainium2 Bass kernel: single-head attention block (B=4, S=2048, E=1024).

Reference (per batch b):
    Q = x@W1+b1; K = x@W2+b2; V = x@W3+b3
    out = softmax(Q K^T / 32) V @ W4 + b4

Algebraic folding (host, fp64, exact):
    scores  = Q K^T = x (W1 W2^T) x^T + u 1^T + 1 v^T + c
      where u_s = x_s.(W1@b2), v_t = x_t.(W2@b1), c = b1.b2.  The u and c
      terms are constant along the key axis, so they cancel exactly in the
      softmax normalization (which this kernel applies on the host) -- only
      the per-key bias v survives, added inside the device exp.
    out     = softmax(.) (x W3 + b3) W4 + b4 = P^ x (W3@W4) + (b3@W4 + b4)
      (softmax rows sum to 1, so b3 folds into the output bias).

Device pipeline per core (core = (batch b, seq-half h), SQ=1024 own queries):
    TT = M^T  XH        [E, SQ]   M = W1@W2^T, XH = x[b]^T own half
    S^T tiles [sk, sq] via lhsT = XT blocks (XT = x[b]^T, full batch -- an
      input, so there are NO collectives), rhs = TT chunks; exp with
      per-partition bias v/32 lands in PX (unnormalized probs)
    sums = 1^T PX       (PE partition-reduce)
    AT = XN^T-blocksT . PX  [E, SQ]  (XN = x[b] natural layout, input)
    RT = W34-blocksT . AT   [E, SQ] -> DRAM (fp32)
Host: out = RT^T * (descale/sums) + b4', where b4' = b3@W4 + b4.

This removes the K/V/output projections AND both AllGathers of the previous
version: 6.45 G MACs/core vs 8.59, zero collectives.

Precision per matmul group (CFG): TT / SC (scores) / A / R each run bf16 or
fp8(e4m3, TRN max +-240) with DoubleRow (2 k-subtiles per instruction, ~1.44x
PE throughput).  fp8 operands are pre-scaled by powers of 2 (host for inputs,
folded into the PSUM->SBUF copy for device-produced tensors); all descaling
folds into the exp scale, the copy scales, and the host normalization.
Default CFG runs scores + attention@x in fp8, TT/R in bf16: simulated
end-to-end rel err 1.5e-2 (gate 2e-2); all-bf16 fallback sims at 1.5e-3.
"""

from contextlib import ExitStack

import ml_dtypes
import numpy as np

import concourse.tile as tile
from concourse import bacc, mybir
from concourse.bass_utils import run_bass_kernel_spmd

BF16 = mybir.dt.bfloat16
F8 = mybir.dt.float8e4
F32 = mybir.dt.float32
AF = mybir.ActivationFunctionType
DR = mybir.MatmulPerfMode.DoubleRow
NP_BF16 = ml_dtypes.bfloat16
NP_F8 = ml_dtypes.float8_e4m3   # TRN-style e4m3: max +-240

B, S, E = 4, 2048, 1024
SQ = S // 2
NCORES = 8
P = 128
NB = 512
ET, ST, QC = E // P, S // P, SQ // NB   # 8, 16, 2

CFG = {"TT": "bf", "SC": "f8", "A": "f8", "R": "bf"}


def _dt(g):
    return F8 if CFG[g] == "f8" else BF16


def emit_folded(tc, aps, exp_scale, tt_scale, a_scale):
    """Per-core program.  tt_scale/a_scale are the PSUM->SBUF copy scales for
    the TT and AT stores; exp_scale multiplies score PSUMs inside the exp."""
    nc = tc.nc
    xt_d, xh_d, xn_d, m_d, w34_d, vb_d, out_d, sums_d = aps
    dt_tt, dt_sc, dt_a, dt_r = _dt("TT"), _dt("SC"), _dt("A"), _dt("R")
    f8_tt, f8_sc, f8_a, f8_r = (CFG[g] == "f8" for g in ("TT", "SC", "A", "R"))

    def r128(ap):  # [(t p), n] -> [t, p, n]
        return ap.rearrange("(t p) n -> t p n", p=P)

    cnt = [0]

    def copy_ps(dst, ps, scale=1.0):
        """PSUM->SBUF copy with optional scale, alternating DVE/ACT."""
        if cnt[0] % 2 == 0:
            if scale == 1.0:
                nc.vector.tensor_copy(dst, ps)
            else:
                nc.vector.tensor_scalar_mul(dst, ps, scale)
        else:
            if scale == 1.0:
                nc.scalar.copy(dst, ps)
            else:
                nc.scalar.activation(dst, ps, AF.Identity, scale=scale)
        cnt[0] += 1

    def mm_group2(pool, stat, mov, nk, f8, finish, tag="ps", prows=P):
        """Both qc chunks share each loaded stationary (halves LDWEIGHTS,
        which DoubleRow can't hide behind FWL).  DoubleRow k-pairs when f8.
        stat(k, w) / mov(k, w, qc) give [128, (w,) cols] slices."""
        pss2 = [pool.tile([prows, NB], F32, name=tag, tag=tag)
                for _ in range(QC)]
        step = 2 if f8 else 1
        for k in range(0, nk, step):
            for qc in range(QC):
                nc.tensor.matmul(pss2[qc][:], stat(k, step), mov(k, step, qc),
                                 start=(k == 0), stop=(k + step >= nk),
                                 perf_mode=DR if f8 else None)
        for qc in range(QC):
            finish(qc, pss2[qc])

    def sl(t, k, w, c0, c1):
        return t[:, k, c0:c1] if w == 1 else t[:, k:k + 2, c0:c1]

    with ExitStack() as ctx:
        pers = ctx.enter_context(tc.tile_pool(name="pers", bufs=1))
        rtp = ctx.enter_context(tc.tile_pool(name="rtp", bufs=3))
        psp = ctx.enter_context(tc.tile_pool(name="psp", bufs=6, space="PSUM"))
        pss = ctx.enter_context(tc.tile_pool(name="pss", bufs=2, space="PSUM"))

        xt_s = pers.tile([P, ET, S], dt_sc, tag="xt")
        xh_s = pers.tile([P, ET, SQ], dt_tt, tag="xh")
        xn_s = pers.tile([P, ST, E], dt_a, tag="xn")
        m_s = pers.tile([P, ET, E], dt_tt, tag="m")
        w34_s = pers.tile([P, ET, E], dt_r, tag="w34")
        vb_s = pers.tile([P, ST], F32, tag="vb")
        tt = pers.tile([P, ET, SQ], dt_sc, tag="tt")
        px = pers.tile([P, ST, SQ], dt_a, tag="px")
        at = pers.tile([P, ET, SQ], dt_r, tag="at")
        ones = pers.tile([P, 2, 16], dt_a, tag="ones")  # 16B ko-step for DR
        sums_sb = pers.tile([1, SQ], F32, tag="sums_sb")

        nc.gpsimd.memset(ones[:], 1.0)
        nc.sync.dma_start(vb_s[:], vb_d)
        # Priming slivers: first TT matmul needs m[:, 0(:2), 0:P] and
        # xh[:, 0(:2), 0:NB]; tiny transfers let the PE start early.
        kw = 2 if f8_tt else 1
        for t in range(kw):
            nc.sync.dma_start(m_s[:, t, 0:P], r128(m_d)[t][:, 0:P])
            nc.sync.dma_start(xh_s[:, t, 0:NB], r128(xh_d)[t][:, 0:NB])
        for t in range(kw):
            nc.sync.dma_start(m_s[:, t, P:], r128(m_d)[t][:, P:])
            nc.sync.dma_start(xh_s[:, t, NB:], r128(xh_d)[t][:, NB:])
        for t in range(kw, ET):
            nc.sync.dma_start(m_s[:, t], r128(m_d)[t])
            nc.sync.dma_start(xh_s[:, t], r128(xh_d)[t])
        for t in range(ET):
            nc.sync.dma_start(xt_s[:, t], r128(xt_d)[t])
        for t in range(ST):
            nc.sync.dma_start(xn_s[:, t], r128(xn_d)[t])
        for t in range(ET):
            nc.sync.dma_start(w34_s[:, t], r128(w34_d)[t])

        def mov(tile_, k, w, qc):
            return sl(tile_, k, w, qc * NB, (qc + 1) * NB)

        # ---- TT = M^T XH  [f, sq] ----
        for ft in range(ET):
            mm_group2(psp,
                      lambda k, w, ft=ft: sl(m_s, k, w, ft * P, (ft + 1) * P),
                      lambda k, w, qc: mov(xh_s, k, w, qc),
                      ET, f8_tt,
                      lambda qc, ps, ft=ft: copy_ps(
                          tt[:, ft, qc * NB:(qc + 1) * NB], ps[:], tt_scale))

        # ---- S^T tiles + exp -> PX ----
        for skt in range(ST):
            mm_group2(psp,
                      lambda k, w, skt=skt: sl(xt_s, k, w, skt * P, (skt + 1) * P),
                      lambda k, w, qc: mov(tt, k, w, qc),
                      ET, f8_sc,
                      lambda qc, ps, skt=skt: nc.scalar.activation(
                          px[:, skt, qc * NB:(qc + 1) * NB], ps[:], AF.Exp,
                          bias=vb_s[:, skt:skt + 1], scale=exp_scale))

        # ---- AT = XN^T-blocks . PX ----
        for ft in range(ET):
            mm_group2(psp,
                      lambda k, w, ft=ft: sl(xn_s, k, w, ft * P, (ft + 1) * P),
                      lambda k, w, qc: mov(px, k, w, qc),
                      ST, f8_a,
                      lambda qc, ps, ft=ft: copy_ps(
                          at[:, ft, qc * NB:(qc + 1) * NB], ps[:], a_scale))

        # ---- sums = 1^T PX ----
        mm_group2(pss,
                  lambda k, w: ones[:, 0:2, 0:1] if w == 2 else ones[:, 0, 0:1],
                  lambda k, w, qc: mov(px, k, w, qc),
                  ST, f8_a,
                  lambda qc, ps: nc.vector.tensor_copy(
                      sums_sb[:, qc * NB:(qc + 1) * NB], ps[:]),
                  tag="pssum", prows=1)

        # ---- RT = W34-blocks^T . AT -> DRAM ----
        def r_finish(qc, ps, gt):
            rt = rtp.tile([P, NB], BF16, name="rt", tag="rt")
            copy_ps(rt[:], ps[:])
            nc.sync.dma_start(
                out_d[gt * P:(gt + 1) * P, qc * NB:(qc + 1) * NB], rt[:])

        for gt in range(ET):
            mm_group2(psp,
                      lambda k, w, gt=gt: sl(w34_s, k, w, gt * P, (gt + 1) * P),
                      lambda k, w, qc: mov(at, k, w, qc),
                      ET, f8_r,
                      lambda qc, ps, gt=gt: r_finish(qc, ps, gt))
        nc.sync.dma_start(sums_d, sums_sb[:])


def build_program(exp_scale, tt_scale, a_scale, num_devices=NCORES, repeats=1):
    nc = bacc.Bacc("TRN2", target_bir_lowering=False, debug=False,
                   num_devices=num_devices)
    aps = (
        nc.dram_tensor("xt", [E, S], _dt("SC"), kind="ExternalInput").ap(),
        nc.dram_tensor("xh", [E, SQ], _dt("TT"), kind="ExternalInput").ap(),
        nc.dram_tensor("xn", [S, E], _dt("A"), kind="ExternalInput").ap(),
        nc.dram_tensor("m", [E, E], _dt("TT"), kind="ExternalInput").ap(),
        nc.dram_tensor("w34", [E, E], _dt("R"), kind="ExternalInput").ap(),
        nc.dram_tensor("vb", [P, ST], F32, kind="ExternalInput").ap(),
        nc.dram_tensor("out", [E, SQ], BF16, kind="ExternalOutput").ap(),
        nc.dram_tensor("sums", [1, SQ], F32, kind="ExternalOutput").ap(),
    )
    with tile.TileContext(nc) as tc:
        for _ in range(repeats):
            emit_folded(tc, aps, exp_scale, tt_scale, a_scale)
    nc.compile()
    return nc


def _pow2_scale(absmax, target=160.0):
    return float(2.0 ** np.floor(np.log2(target / max(absmax, 1e-30))))


def _cast(a, group, scale):
    if CFG[group] == "bf":
        return np.ascontiguousarray(a.astype(NP_BF16))
    return np.ascontiguousarray(
        np.clip(a * scale, -240.0, 240.0).astype(NP_F8))


def prep(x, W1, b1, W2, b2, W3, b3, W4, b4):
    """Host folds + scales + per-core in_maps.  Returns (in_maps, consts)."""
    M = (W1.astype(np.float64) @ W2.astype(np.float64).T).astype(np.float32)
    W34 = (W3.astype(np.float64) @ W4.astype(np.float64)).astype(np.float32)
    w2b1 = (W2.astype(np.float64) @ b1.astype(np.float64))
    v = (x.astype(np.float64).reshape(-1, E) @ w2b1).astype(np.float32)
    v = v.reshape(B, S)
    b4p = (b3.astype(np.float64) @ W4.astype(np.float64) + b4).astype(np.float32)

    sxt = _pow2_scale(np.abs(x).max()) if CFG["SC"] == "f8" else 1.0
    sxh = _pow2_scale(np.abs(x).max()) if CFG["TT"] == "f8" else 1.0
    sxn = _pow2_scale(np.abs(x).max()) if CFG["A"] == "f8" else 1.0
    sM = _pow2_scale(np.abs(M).max()) if CFG["TT"] == "f8" else 1.0
    sW34 = _pow2_scale(np.abs(W34).max()) if CFG["R"] == "f8" else 1.0
    if CFG["SC"] == "f8":
        # TT absmax from a row sample (TT is computed on device); pow2 scale
        # with 2x headroom to +-240 absorbs the sampling error.
        samp = x.reshape(-1, E)[:: (B * S) // 256][:256].astype(np.float32)
        est = np.abs(samp @ M).max() * 1.15
        sTT = _pow2_scale(est, target=110.0)
    else:
        sTT = 1.0
    sA = 1.0   # A stored bf16 in all supported configs

    exp_scale = 1.0 / (32.0 * sxt * sTT)
    tt_scale = sTT / (sM * sxh)
    a_scale = sA / sxn
    rdesc = 1.0 / (np.float64(sW34) * sA)

    ws = {"m": _cast(M, "TT", sM), "w34": _cast(W34, "R", sW34)}
    in_maps = []
    for i in range(NCORES):
        b, h = divmod(i, 2)
        xTb = x[b].T
        in_maps.append({
            "xt": _cast(xTb, "SC", sxt),
            "xh": _cast(xTb[:, h * SQ:(h + 1) * SQ], "TT", sxh),
            "xn": _cast(x[b], "A", sxn),
            "vb": np.ascontiguousarray(
                (v[b] / 32.0).reshape(ST, P).T.astype(np.float32)),
            **ws,
        })
    return in_maps, (exp_scale, tt_scale, a_scale, rdesc, b4p)


_PROGRAMS = {}
_LAST_CONSTS = None


def make_in_maps(x, W1, b1, W2, b2, W3, b3, W4, b4):
    """test.py entry point; also records consts for build_program()."""
    global _LAST_CONSTS
    args = (np.asarray(a, np.float32)
            for a in (x, W1, b1, W2, b2, W3, b3, W4, b4))
    in_maps, consts = prep(*args)
    _LAST_CONSTS = consts
    return in_maps


def get_program(exp_scale, tt_scale, a_scale, repeats=1):
    key = (exp_scale, tt_scale, a_scale, repeats)
    if key not in _PROGRAMS:
        _PROGRAMS[key] = build_program(exp_scale, tt_scale, a_scale,
                                       repeats=repeats)
    return _PROGRAMS[key]


def kernel(x, W1, b1, W2, b2, W3, b3, W4, b4):
    args = [np.asarray(a, np.float32)
            for a in (x, W1, b1, W2, b2, W3, b3, W4, b4)]
    in_maps, (exp_scale, tt_scale, a_scale, rdesc, b4p) = prep(*args)
    nc = get_program(exp_scale, tt_scale, a_scale)
    res = run_bass_kernel_spmd(nc, in_maps, core_ids=list(range(NCORES)))
    out = np.empty((B, S, E), np.float32)
    for i in range(NCORES):
        b, h = divmod(i, 2)
        rt = res.results[i]["out"].astype(np.float32)   # [E,SQ] = R^T*sW34*sA
        sums = res.results[i]["sums"][0]    # [SQ]
        dst = out[b, h * SQ:(h + 1) * SQ, :]
        np.multiply(rt.T, (rdesc / sums)[:, None].astype(np.float32), out=dst)
        dst += b4p[None, :]
    return out


# revision 17
# speedup vs baseline: 1.0442x; 1.0442x over previous
"""Trainium2 Bass kernel: single-head attention block (B=4, S=2048, E=1024).

Reference (per batch b):
    Q = x@W1+b1; K = x@W2+b2; V = x@W3+b3
    out = softmax(Q K^T / 32) V @ W4 + b4

Algebraic folding (host, fp64, exact):
    scores  = Q K^T = x (W1 W2^T) x^T + u 1^T + 1 v^T + c
      where u_s = x_s.(W1@b2), v_t = x_t.(W2@b1), c = b1.b2.  The u and c
      terms are constant along the key axis, so they cancel exactly in the
      softmax normalization (which this kernel applies on the host) -- only
      the per-key bias v survives, added inside the device exp.
    out     = softmax(.) (x W3 + b3) W4 + b4 = P^ x (W3@W4) + (b3@W4 + b4)
      (softmax rows sum to 1, so b3 folds into the output bias).

Device pipeline per core (core = (batch b, seq-half h), SQ=1024 own queries):
    TT = M^T  XH        [E, SQ]   M = W1@W2^T, XH = x[b]^T own half
    S^T tiles [sk, sq] via lhsT = XT blocks (XT = x[b]^T, full batch -- an
      input, so there are NO collectives), rhs = TT chunks; exp with
      per-partition bias v/32 lands in PX (unnormalized probs)
    sums = 1^T PX       (PE partition-reduce)
    AT = XN^T-blocksT . PX  [E, SQ]  (XN = x[b] natural layout, input)
    RT = W34-blocksT . AT   [E, SQ] -> DRAM (fp32)
Host: out = RT^T * (descale/sums) + b4', where b4' = b3@W4 + b4.

This removes the K/V/output projections AND both AllGathers of the previous
version: 6.45 G MACs/core vs 8.59, zero collectives.

Precision per matmul group (CFG): TT / SC (scores) / A / R each run bf16 or
fp8(e4m3, TRN max +-240) with DoubleRow (2 k-subtiles per instruction, ~1.44x
PE throughput).  fp8 operands are pre-scaled by powers of 2 (host for inputs,
folded into the PSUM->SBUF copy for device-produced tensors); all descaling
folds into the exp scale, the copy scales, and the host normalization.
Default CFG runs scores + attention@x in fp8, TT/R in bf16: simulated
end-to-end rel err 1.5e-2 (gate 2e-2); all-bf16 fallback sims at 1.5e-3.
"""

from contextlib import ExitStack

import ml_dtypes
import numpy as np

import concourse.tile as tile
from concourse import bacc, mybir
from concourse.bass_utils import run_bass_kernel_spmd

BF16 = mybir.dt.bfloat16
F8 = mybir.dt.float8e4
F32 = mybir.dt.float32
AF = mybir.ActivationFunctionType
DR = mybir.MatmulPerfMode.DoubleRow
NP_BF16 = ml_dtypes.bfloat16
NP_F8 = ml_dtypes.float8_e4m3   # TRN-style e4m3: max +-240

B, S, E = 4, 2048, 1024
SQ = S // 2
NCORES = 8
P = 128
NB = 512
ET, ST, QC = E // P, S // P, SQ // NB   # 8, 16, 2

CFG = {"TT": "bf", "SC": "f8", "A": "f8", "R": "bf"}


def _dt(g):
    return F8 if CFG[g] == "f8" else BF16


def emit_folded(tc, aps, exp_scale, tt_scale, a_scale):
    """Per-core program.  tt_scale/a_scale are the PSUM->SBUF copy scales for
    the TT and AT stores; exp_scale multiplies score PSUMs inside the exp."""
    nc = tc.nc
    xt_d, xh_d, xn_d, m_d, w34_d, vb_d, out_d, sums_d = aps
    dt_tt, dt_sc, dt_a, dt_r = _dt("TT"), _dt("SC"), _dt("A"), _dt("R")
    f8_tt, f8_sc, f8_a, f8_r = (CFG[g] == "f8" for g in ("TT", "SC", "A", "R"))

    def r128(ap):  # [(t p), n] -> [t, p, n]
        return ap.rearrange("(t p) n -> t p n", p=P)

    cnt = [0]

    def copy_ps(dst, ps, scale=1.0):
        """PSUM->SBUF copy with optional scale, alternating DVE/ACT."""
        if cnt[0] % 2 == 0:
            if scale == 1.0:
                nc.vector.tensor_copy(dst, ps)
            else:
                nc.vector.tensor_scalar_mul(dst, ps, scale)
        else:
            if scale == 1.0:
                nc.scalar.copy(dst, ps)
            else:
                nc.scalar.activation(dst, ps, AF.Identity, scale=scale)
        cnt[0] += 1

    def sl(t, k, w, c0, c1):
        return t[:, k, c0:c1] if w == 1 else t[:, k:k + 2, c0:c1]

    with ExitStack() as ctx:
        pers = ctx.enter_context(tc.tile_pool(name="pers", bufs=1))
        rtp = ctx.enter_context(tc.tile_pool(name="rtp", bufs=3))
        psp = ctx.enter_context(tc.tile_pool(name="psp", bufs=6, space="PSUM"))
        pss = ctx.enter_context(tc.tile_pool(name="pss", bufs=2, space="PSUM"))

        xt_s = pers.tile([P, ET, S], dt_sc, tag="xt")
        xh_s = pers.tile([P, ET, SQ], dt_tt, tag="xh")
        xn_s = pers.tile([P, ST, E], dt_a, tag="xn")
        m_s = pers.tile([P, ET, E], dt_tt, tag="m")
        w34_s = pers.tile([P, ET, E], dt_r, tag="w34")
        vb_s = pers.tile([P, ST], F32, tag="vb")
        tt = pers.tile([P, ET, SQ], dt_sc, tag="tt")
        px = pers.tile([P, ST, SQ], dt_a, tag="px")
        at = pers.tile([P, ET, SQ], dt_r, tag="at")
        ones = pers.tile([P, 2, 16], dt_a, tag="ones")  # 16B ko-step for DR
        sums_sb = pers.tile([1, SQ], F32, tag="sums_sb")

        nc.gpsimd.memset(ones[:], 1.0)
        nc.sync.dma_start(vb_s[:], vb_d)
        # Priming slivers: first TT matmul needs m[:, 0(:2), 0:P] and
        # xh[:, 0(:2), 0:NB]; tiny transfers let the PE start early.
        kw = 2 if f8_tt else 1
        for t in range(kw):
            nc.sync.dma_start(m_s[:, t, 0:P], r128(m_d)[t][:, 0:P])
            nc.sync.dma_start(xh_s[:, t, 0:NB], r128(xh_d)[t][:, 0:NB])
        for t in range(kw):
            nc.sync.dma_start(m_s[:, t, P:], r128(m_d)[t][:, P:])
            nc.sync.dma_start(xh_s[:, t, NB:], r128(xh_d)[t][:, NB:])
        for t in range(kw, ET):
            nc.sync.dma_start(m_s[:, t], r128(m_d)[t])
            nc.sync.dma_start(xh_s[:, t], r128(xh_d)[t])
        for t in range(ET):
            nc.sync.dma_start(xt_s[:, t], r128(xt_d)[t])
        for t in range(ST):
            nc.sync.dma_start(xn_s[:, t], r128(xn_d)[t])
        for t in range(ET):
            nc.sync.dma_start(w34_s[:, t], r128(w34_d)[t])

        def mm_acc(ps, stat, mov, nk, f8):
            step = 2 if f8 else 1
            for k in range(0, nk, step):
                nc.tensor.matmul(ps[:], stat(k, step), mov(k, step),
                                 start=(k == 0), stop=(k + step >= nk),
                                 perf_mode=DR if f8 else None)

        # ---- TT = M^T XH  [f, sq] ----
        for qc in range(QC):
            for ft in range(ET):
                ps = psp.tile([P, NB], F32, name="ps", tag="ps")
                mm_acc(ps,
                       lambda k, w, ft=ft: sl(m_s, k, w, ft * P, (ft + 1) * P),
                       lambda k, w, qc=qc: sl(xh_s, k, w, qc * NB, (qc + 1) * NB),
                       ET, f8_tt)
                copy_ps(tt[:, ft, qc * NB:(qc + 1) * NB], ps[:], tt_scale)

        # ---- scores + exp (per qc), then A, sums, R ----
        def sc_chunk(qc):
            for skt in range(ST):
                ps = psp.tile([P, NB], F32, name="ps", tag="ps")
                mm_acc(ps,
                       lambda k, w, skt=skt: sl(xt_s, k, w, skt * P, (skt + 1) * P),
                       lambda k, w, qc=qc: sl(tt, k, w, qc * NB, (qc + 1) * NB),
                       ET, f8_sc)
                nc.scalar.activation(px[:, skt, qc * NB:(qc + 1) * NB], ps[:],
                                     AF.Exp, bias=vb_s[:, skt:skt + 1],
                                     scale=exp_scale)

        def a_chunk(qc):
            for ft in range(ET):
                ps = psp.tile([P, NB], F32, name="ps", tag="ps")
                mm_acc(ps,
                       lambda k, w, ft=ft: sl(xn_s, k, w, ft * P, (ft + 1) * P),
                       lambda k, w, qc=qc: sl(px, k, w, qc * NB, (qc + 1) * NB),
                       ST, f8_a)
                copy_ps(at[:, ft, qc * NB:(qc + 1) * NB], ps[:], a_scale)

        def sums_chunk(qc):
            ps = pss.tile([1, NB], F32, name="pssum", tag="pssum")
            mm_acc(ps,
                   lambda k, w: ones[:, 0:2, 0:1] if w == 2 else ones[:, 0, 0:1],
                   lambda k, w, qc=qc: sl(px, k, w, qc * NB, (qc + 1) * NB),
                   ST, f8_a)
            nc.vector.tensor_copy(sums_sb[:, qc * NB:(qc + 1) * NB], ps[:])

        def r_chunk(qc):
            for gt in range(ET):
                ps = psp.tile([P, NB], F32, name="ps", tag="ps")
                mm_acc(ps,
                       lambda k, w, gt=gt: sl(w34_s, k, w, gt * P, (gt + 1) * P),
                       lambda k, w, qc=qc: sl(at, k, w, qc * NB, (qc + 1) * NB),
                       ET, f8_r)
                rt = rtp.tile([P, NB], BF16, name="rt", tag="rt")
                copy_ps(rt[:], ps[:])
                nc.sync.dma_start(
                    out_d[gt * P:(gt + 1) * P, qc * NB:(qc + 1) * NB], rt[:])

        sc_chunk(0)
        sc_chunk(1)
        a_chunk(0)
        sums_chunk(0)
        a_chunk(1)
        sums_chunk(1)
        r_chunk(0)
        r_chunk(1)
        nc.sync.dma_start(sums_d, sums_sb[:])


def build_program(exp_scale, tt_scale, a_scale, num_devices=NCORES, repeats=1):
    nc = bacc.Bacc("TRN2", target_bir_lowering=False, debug=False,
                   num_devices=num_devices)
    aps = (
        nc.dram_tensor("xt", [E, S], _dt("SC"), kind="ExternalInput").ap(),
        nc.dram_tensor("xh", [E, SQ], _dt("TT"), kind="ExternalInput").ap(),
        nc.dram_tensor("xn", [S, E], _dt("A"), kind="ExternalInput").ap(),
        nc.dram_tensor("m", [E, E], _dt("TT"), kind="ExternalInput").ap(),
        nc.dram_tensor("w34", [E, E], _dt("R"), kind="ExternalInput").ap(),
        nc.dram_tensor("vb", [P, ST], F32, kind="ExternalInput").ap(),
        nc.dram_tensor("out", [E, SQ], BF16, kind="ExternalOutput").ap(),
        nc.dram_tensor("sums", [1, SQ], F32, kind="ExternalOutput").ap(),
    )
    with tile.TileContext(nc) as tc:
        for _ in range(repeats):
            emit_folded(tc, aps, exp_scale, tt_scale, a_scale)
    nc.compile()
    return nc


def _pow2_scale(absmax, target=160.0):
    return float(2.0 ** np.floor(np.log2(target / max(absmax, 1e-30))))


def _cast(a, group, scale):
    if CFG[group] == "bf":
        return np.ascontiguousarray(a.astype(NP_BF16))
    return np.ascontiguousarray(
        np.clip(a * scale, -240.0, 240.0).astype(NP_F8))


def prep(x, W1, b1, W2, b2, W3, b3, W4, b4):
    """Host folds + scales + per-core in_maps.  Returns (in_maps, consts)."""
    M = (W1.astype(np.float64) @ W2.astype(np.float64).T).astype(np.float32)
    W34 = (W3.astype(np.float64) @ W4.astype(np.float64)).astype(np.float32)
    w2b1 = (W2.astype(np.float64) @ b1.astype(np.float64))
    v = (x.astype(np.float64).reshape(-1, E) @ w2b1).astype(np.float32)
    v = v.reshape(B, S)
    b4p = (b3.astype(np.float64) @ W4.astype(np.float64) + b4).astype(np.float32)

    sxt = _pow2_scale(np.abs(x).max()) if CFG["SC"] == "f8" else 1.0
    sxh = _pow2_scale(np.abs(x).max()) if CFG["TT"] == "f8" else 1.0
    sxn = _pow2_scale(np.abs(x).max()) if CFG["A"] == "f8" else 1.0
    sM = _pow2_scale(np.abs(M).max()) if CFG["TT"] == "f8" else 1.0
    sW34 = _pow2_scale(np.abs(W34).max()) if CFG["R"] == "f8" else 1.0
    if CFG["SC"] == "f8":
        # TT absmax from a row sample (TT is computed on device); pow2 scale
        # with 2x headroom to +-240 absorbs the sampling error.
        samp = x.reshape(-1, E)[:: (B * S) // 256][:256].astype(np.float32)
        est = np.abs(samp @ M).max() * 1.15
        sTT = _pow2_scale(est, target=110.0)
    else:
        sTT = 1.0
    sA = 1.0   # A stored bf16 in all supported configs

    exp_scale = 1.0 / (32.0 * sxt * sTT)
    tt_scale = sTT / (sM * sxh)
    a_scale = sA / sxn
    rdesc = 1.0 / (np.float64(sW34) * sA)

    ws = {"m": _cast(M, "TT", sM), "w34": _cast(W34, "R", sW34)}
    in_maps = []
    for i in range(NCORES):
        b, h = divmod(i, 2)
        xTb = x[b].T
        in_maps.append({
            "xt": _cast(xTb, "SC", sxt),
            "xh": _cast(xTb[:, h * SQ:(h + 1) * SQ], "TT", sxh),
            "xn": _cast(x[b], "A", sxn),
            "vb": np.ascontiguousarray(
                (v[b] / 32.0).reshape(ST, P).T.astype(np.float32)),
            **ws,
        })
    return in_maps, (exp_scale, tt_scale, a_scale, rdesc, b4p)


_PROGRAMS = {}
_LAST_CONSTS = None


def make_in_maps(x, W1, b1, W2, b2, W3, b3, W4, b4):
    """test.py entry point; also records consts for build_program()."""
    global _LAST_CONSTS
    args = (np.asarray(a, np.float32)
            for a in (x, W1, b1, W2, b2, W3, b3, W4, b4))
    in_maps, consts = prep(*args)
    _LAST_CONSTS = consts
    return in_maps


def get_program(exp_scale, tt_scale, a_scale, repeats=1):
    key = (exp_scale, tt_scale, a_scale, repeats)
    if key not in _PROGRAMS:
        _PROGRAMS[key] = build_program(exp_scale, tt_scale, a_scale,
                                       repeats=repeats)
    return _PROGRAMS[key]


def kernel(x, W1, b1, W2, b2, W3, b3, W4, b4):
    args = [np.asarray(a, np.float32)
            for a in (x, W1, b1, W2, b2, W3, b3, W4, b4)]
    in_maps, (exp_scale, tt_scale, a_scale, rdesc, b4p) = prep(*args)
    nc = get_program(exp_scale, tt_scale, a_scale)
    res = run_bass_kernel_spmd(nc, in_maps, core_ids=list(range(NCORES)))
    out = np.empty((B, S, E), np.float32)
    for i in range(NCORES):
        b, h = divmod(i, 2)
        rt = res.results[i]["out"].astype(np.float32)   # [E,SQ] = R^T*sW34*sA
        sums = res.results[i]["sums"][0]    # [SQ]
        dst = out[b, h * SQ:(h + 1) * SQ, :]
        np.multiply(rt.T, (rdesc / sums)[:, None].astype(np.float32), out=dst)
        dst += b4p[None, :]
    return out
